# revision 8
# baseline (speedup 1.0000x reference)
"""Trainium2 Bass kernel for the vq_codebook (dual-modality Sinkhorn VQ) problem.

Data-parallel over batch: 8 NeuronCores x 2 batch elements each, both
modalities per core.  Per (batch, modality) unit:
  sim = l2norm(x) @ l2norm(proto).T            (PE, fp32)
  a   = sinkhorn(sim)  in u/v scaling-vector form: the full q matrix is
        never re-materialized per iteration; row sums come from one DVE
        multiply pass + one ACT copy-with-accum pass, column sums from a
        PE matvec with the row-scaling vector as the stationary operand.
  z   = a @ l2norm(proto)                      (PE, q0^T stationary)
sem_consistency is reduced on host from the assignment column sums.
"""
import sys

sys.path.insert(0, "/opt/trn_rl_repo")

import numpy as np

import concourse.bass as bass
import concourse.bacc as bacc
import concourse.tile as tile
from concourse import mybir
from concourse.bass_utils import run_bass_kernel_spmd
from concourse.masks import make_identity

B, N, D, K = 16, 1024, 256, 256
NCORES = 8
BPC = B // NCORES          # batches per core
EPS = 1e-8
TAU = 0.05
ITERS = 5
NT = N // 128              # 8 n-blocks
KC = K // 128              # 2 k-chunks
DC = D // 128              # 2 d-chunks
f32 = mybir.dt.float32
ACT_COPY = mybir.ActivationFunctionType.Copy
ACT_EXP = mybir.ActivationFunctionType.Exp
MULT = mybir.AluOpType.mult
ADD = mybir.AluOpType.add


def _build():
    nc = bacc.Bacc("TRN2", target_bir_lowering=False)

    xin, pin, sim_o, a_o, z_o = {}, {}, {}, {}, {}
    for mod in ("rgb", "sn"):
        xin[mod] = nc.declare_dram_parameter(f"x_{mod}", [BPC, N, D], f32, False)
        pin[mod] = nc.declare_dram_parameter(f"p_{mod}", [K, D], f32, False)
    for mod in ("rgb", "sn"):
        sim_o[mod] = nc.declare_dram_parameter(f"sim_{mod}", [BPC, N, K], f32, True)
        a_o[mod] = nc.declare_dram_parameter(f"a_{mod}", [BPC, N, K], f32, True)
        z_o[mod] = nc.declare_dram_parameter(f"z_{mod}", [BPC, N, D], f32, True)

    with tile.TileContext(nc) as tc:
        with (
            tc.tile_pool(name="const", bufs=1) as constp,
            tc.tile_pool(name="proto", bufs=1) as protop,
            tc.tile_pool(name="big", bufs=2) as bigp,
            tc.tile_pool(name="small", bufs=3) as smallp,
            tc.tile_pool(name="ps_tp", bufs=2, space="PSUM") as ps_tp,
            tc.tile_pool(name="ps_mm", bufs=2, space="PSUM") as ps_mm,
            tc.tile_pool(name="ps_vb", bufs=1, space="PSUM") as ps_vb,
            tc.tile_pool(name="ps_mv", bufs=2, space="PSUM") as ps_mv,
        ):
            ident = constp.tile([128, 128], f32)
            make_identity(nc, ident[:, :])
            ones_row = constp.tile([1, 128], f32)
            nc.vector.memset(ones_row[:, :], 1.0)
            ones_col = constp.tile([128, 1], f32)
            nc.vector.memset(ones_col[:, :], 1.0)

            # warm PE's vector clock on ident's producer so later transposes
            # never need more than one semaphore wait on their LDWEIGHTS
            ps_warm = ps_mv.tile([128, 1], f32, tag="mv")
            nc.tensor.matmul(
                ps_warm[:, :], ident[:, :], ident[:, 0:1], start=True, stop=True
            )

            # ---- prototype tables: load, l2-normalize, transpose ----
            p_norm, p_T = {}, {}
            for mod in ("rgb", "sn"):
                pt = protop.tile([128, KC, D], f32, tag=f"p_{mod}")
                for kk in range(KC):
                    nc.sync.dma_start(
                        out=pt[:, kk, :], in_=pin[mod][kk * 128:(kk + 1) * 128, :]
                    )
                ssp = smallp.tile([128, KC], f32)
                for kk in range(KC):
                    scr = smallp.tile([128, D], f32, tag="psq_scr")
                    nc.vector.tensor_mul(
                        out=scr[:, :], in0=pt[:, kk, :], in1=pt[:, kk, :]
                    )
                    scr2 = smallp.tile([128, D], f32, tag="psq_scr2")
                    nc.scalar.activation(
                        out=scr2[:, :], in_=scr[:, :], func=ACT_COPY,
                        accum_out=ssp[:, kk:kk + 1],
                    )
                nrm = smallp.tile([128, KC], f32)
                nc.scalar.sqrt(out=nrm[:, :], in_=ssp[:, :])
                nrm2 = smallp.tile([128, KC], f32)
                nc.vector.tensor_scalar_add(
                    out=nrm2[:, :], in0=nrm[:, :], scalar1=EPS
                )
                rp = smallp.tile([128, KC], f32)
                nc.vector.reciprocal(out=rp[:, :], in_=nrm2[:, :])
                pn = protop.tile([128, KC, D], f32, tag=f"pn_{mod}")
                for kk in range(KC):
                    nc.vector.tensor_scalar_mul(
                        out=pn[:, kk, :], in0=pt[:, kk, :], scalar1=rp[:, kk:kk + 1]
                    )
                pT = protop.tile([128, DC, K], f32, tag=f"pT_{mod}")
                for dj in range(DC):
                    pst = ps_tp.tile([128, 512], f32, tag="tp")
                    nc.tensor.matmul(
                        pst[:, 0:1], pn[:, 0, 0:128], pn[:, 0, 0:1],
                        start=True, stop=True,
                    )
                    for kk in range(KC):
                        nc.tensor.transpose(
                            pst[:, kk * 128:(kk + 1) * 128],
                            pn[:, kk, dj * 128:(dj + 1) * 128],
                            ident[:, :],
                        )
                    for kk in range(KC):
                        nc.scalar.copy(
                            out=pT[:, dj, kk * 128:(kk + 1) * 128],
                            in_=pst[:, kk * 128:(kk + 1) * 128],
                        )
                p_norm[mod] = pn
                p_T[mod] = pT

            # ---- per (modality, batch) unit ----
            for mod in ("rgb", "sn"):
                for b in range(BPC):
                    xdram = xin[mod][b].rearrange("(i p) d -> p i d", p=128)
                    xt = bigp.tile([128, NT, D], f32, tag="xt")
                    nc.sync.dma_start(out=xt[:, :, :], in_=xdram)

                    # row norms: rx = 1/(||x_n|| + eps), rx20 = rx/tau
                    ss = smallp.tile([128, NT], f32, tag="ss")
                    for i in range(NT):
                        scr = smallp.tile([128, D], f32, tag="xsq_scr")
                        nc.vector.tensor_mul(
                            out=scr[:, :], in0=xt[:, i, :], in1=xt[:, i, :]
                        )
                        scr2 = smallp.tile([128, D], f32, tag="xsq_scr2")
                        nc.scalar.activation(
                            out=scr2[:, :], in_=scr[:, :], func=ACT_COPY,
                            accum_out=ss[:, i:i + 1],
                        )
                    nrm = smallp.tile([128, NT], f32, tag="xnrm")
                    nc.scalar.sqrt(out=nrm[:, :], in_=ss[:, :])
                    nrm2 = smallp.tile([128, NT], f32, tag="xnrm2")
                    nc.vector.tensor_scalar_add(
                        out=nrm2[:, :], in0=nrm[:, :], scalar1=EPS
                    )
                    rx = smallp.tile([128, NT], f32, tag="rx")
                    nc.vector.reciprocal(out=rx[:, :], in_=nrm2[:, :])
                    rx20 = smallp.tile([128, NT], f32, tag="rx20")
                    nc.vector.tensor_scalar_mul(
                        out=rx20[:, :], in0=rx[:, :], scalar1=1.0 / TAU
                    )

                    # transpose raw x -> xT [d, n]
                    xT = bigp.tile([128, DC, N], f32, tag="xT")
                    for dj in range(DC):
                        for g4 in range(NT // 4):
                            pst = ps_tp.tile([128, 512], f32, tag="tp")
                            nc.tensor.matmul(
                                pst[:, 0:1], xt[:, 0, 0:128], xt[:, 0, 0:1],
                                start=True, stop=True,
                            )
                            for ii in range(4):
                                i = g4 * 4 + ii
                                nc.tensor.transpose(
                                    pst[:, ii * 128:(ii + 1) * 128],
                                    xt[:, i, dj * 128:(dj + 1) * 128],
                                    ident[:, :],
                                )
                            nc.scalar.copy(
                                out=xT[:, dj, g4 * 512:(g4 + 1) * 512],
                                in_=pst[:, :],
                            )

                    # sim (raw) = xT.T @ pT ; scale rows by rx going out;
                    # q0 = exp(sim/tau) with fused per-row scale + row sums
                    sim_sb = bigp.tile([128, NT, K], f32, tag="sim_sb")
                    q0 = bigp.tile([128, NT, K], f32, tag="q0")
                    w0 = smallp.tile([128, NT], f32, tag="w0")
                    for i in range(NT):
                        ps = ps_mm.tile([128, K], f32, tag="mm")
                        for dj in range(DC):
                            nc.tensor.matmul(
                                ps[:, :],
                                xT[:, dj, i * 128:(i + 1) * 128],
                                p_T[mod][:, dj, :],
                                start=(dj == 0), stop=(dj == DC - 1),
                            )
                        nc.scalar.activation(
                            out=sim_sb[:, i, :], in_=ps[:, :], func=ACT_COPY,
                            scale=rx[:, i:i + 1],
                        )
                        nc.scalar.activation(
                            out=q0[:, i, :], in_=ps[:, :], func=ACT_EXP,
                            scale=rx20[:, i:i + 1],
                            accum_out=w0[:, i:i + 1],
                        )
                    nc.sync.dma_start(
                        out=sim_o[mod][b].rearrange("(i p) k -> p i k", p=128),
                        in_=sim_sb[:, :, :],
                    )

                    # q0T via PE transposes (for the z matmul)
                    q0T = bigp.tile([128, KC, N], f32, tag="q0T")
                    for kk in range(KC):
                        for g4 in range(NT // 4):
                            pst = ps_tp.tile([128, 512], f32, tag="tp")
                            nc.tensor.matmul(
                                pst[:, 0:1], q0[:, 0, 0:128], q0[:, 0, 0:1],
                                start=True, stop=True,
                            )
                            for ii in range(4):
                                i = g4 * 4 + ii
                                nc.tensor.transpose(
                                    pst[:, ii * 128:(ii + 1) * 128],
                                    q0[:, i, kk * 128:(kk + 1) * 128],
                                    ident[:, :],
                                )
                            nc.scalar.copy(
                                out=q0T[:, kk, g4 * 512:(g4 + 1) * 512],
                                in_=pst[:, :],
                            )

                    # global sum S = sum(w0); g = 1/(S+eps); gb = bcast(g)
                    ps_s = ps_mv.tile([1, NT], f32, tag="mv")
                    nc.tensor.matmul(
                        ps_s[:, :], ones_col[:, :], w0[:, :], start=True, stop=True
                    )
                    s_row = smallp.tile([1, NT], f32, tag="s_row")
                    nc.scalar.copy(out=s_row[:, :], in_=ps_s[:, :])
                    ps_s8 = ps_mv.tile([NT, 1], f32, tag="mv")
                    nc.tensor.transpose(
                        ps_s8[:, :], s_row[:, :], ident[0:1, 0:1]
                    )
                    s_col = smallp.tile([NT, 1], f32, tag="s_col")
                    nc.scalar.copy(out=s_col[:, :], in_=ps_s8[:, :])
                    ps_S = ps_mv.tile([1, 1], f32, tag="mv")
                    nc.tensor.matmul(
                        ps_S[:, :], ones_col[0:NT, :], s_col[:, :],
                        start=True, stop=True,
                    )
                    Se = smallp.tile([1, 1], f32, tag="Se")
                    nc.vector.tensor_scalar_add(
                        out=Se[:, :], in0=ps_S[:, :], scalar1=EPS
                    )
                    g = smallp.tile([1, 1], f32, tag="g")
                    nc.vector.reciprocal(out=g[:, :], in_=Se[:, :])
                    ps_gb = ps_mv.tile([128, 1], f32, tag="mv")
                    nc.tensor.matmul(
                        ps_gb[:, :], ones_row[:, :], g[:, :], start=True, stop=True
                    )
                    gb = smallp.tile([128, 1], f32, tag="gb")
                    nc.scalar.copy(out=gb[:, :], in_=ps_gb[:, :])

                    # U1 = g*(1/N) / (g*w0 + eps)
                    t0 = smallp.tile([128, NT], f32, tag="t0")
                    nc.vector.tensor_scalar(
                        out=t0[:, :], in0=w0[:, :], scalar1=gb[:, :], scalar2=EPS,
                        op0=MULT, op1=ADD,
                    )
                    tr = smallp.tile([128, NT], f32, tag="tr")
                    nc.vector.reciprocal(out=tr[:, :], in_=t0[:, :])
                    U = smallp.tile([128, NT], f32, tag="U")
                    nc.vector.tensor_scalar(
                        out=U[:, :], in0=tr[:, :], scalar1=gb[:, :],
                        scalar2=1.0 / N, op0=MULT, op1=MULT,
                    )

                    V = None
                    vb_sb = None
                    qs = None
                    w_cur = w0
                    for it in range(ITERS):
                        if it > 0:
                            # U <- U * (1/N) / (U*w + eps)
                            t1 = smallp.tile([128, NT], f32, tag="t1")
                            nc.vector.tensor_mul(
                                out=t1[:, :], in0=U[:, :], in1=w_cur[:, :]
                            )
                            t2 = smallp.tile([128, NT], f32, tag="t2")
                            nc.vector.tensor_scalar_add(
                                out=t2[:, :], in0=t1[:, :], scalar1=EPS
                            )
                            t3 = smallp.tile([128, NT], f32, tag="t3")
                            nc.vector.reciprocal(out=t3[:, :], in_=t2[:, :])
                            t4 = smallp.tile([128, NT], f32, tag="t4")
                            nc.vector.tensor_scalar_mul(
                                out=t4[:, :], in0=t3[:, :], scalar1=1.0 / N
                            )
                            Un = smallp.tile([128, NT], f32, tag="U")
                            nc.vector.tensor_mul(
                                out=Un[:, :], in0=U[:, :], in1=t4[:, :]
                            )
                            U = Un

                        # column sums m[k] = sum_n q0[n,k] * U[n]
                        ps_m = ps_mv.tile([1, K], f32, tag="mv")
                        for i in range(NT):
                            nc.tensor.matmul(
                                ps_m[:, :], U[:, i:i + 1], q0[:, i, :],
                                start=(i == 0), stop=(i == NT - 1),
                            )
                        # V update: V <- V*(1/K)/(V*m + eps)   (V0 = 1)
                        if it == 0:
                            tv = smallp.tile([1, K], f32, tag="tv")
                            nc.vector.tensor_scalar_add(
                                out=tv[:, :], in0=ps_m[:, :], scalar1=EPS
                            )
                            tvr = smallp.tile([1, K], f32, tag="tvr")
                            nc.vector.reciprocal(out=tvr[:, :], in_=tv[:, :])
                            Vn = smallp.tile([1, K], f32, tag="V")
                            nc.vector.tensor_scalar_mul(
                                out=Vn[:, :], in0=tvr[:, :], scalar1=1.0 / K
                            )
                        else:
                            tv0 = smallp.tile([1, K], f32, tag="tv0")
                            nc.vector.tensor_mul(
                                out=tv0[:, :], in0=ps_m[:, :], in1=V[:, :]
                            )
                            tv = smallp.tile([1, K], f32, tag="tv")
                            nc.vector.tensor_scalar_add(
                                out=tv[:, :], in0=tv0[:, :], scalar1=EPS
                            )
                            tvr = smallp.tile([1, K], f32, tag="tvr")
                            nc.vector.reciprocal(out=tvr[:, :], in_=tv[:, :])
                            tvs = smallp.tile([1, K], f32, tag="tvs")
                            nc.vector.tensor_scalar_mul(
                                out=tvs[:, :], in0=tvr[:, :], scalar1=1.0 / K
                            )
                            Vn = smallp.tile([1, K], f32, tag="V")
                            nc.vector.tensor_mul(
                                out=Vn[:, :], in0=tvs[:, :], in1=V[:, :]
                            )
                        V = Vn

                        # broadcast V down the partitions (exact fp32 matmul)
                        ps_b = ps_vb.tile([128, K], f32, tag="vb")
                        nc.tensor.matmul(
                            ps_b[:, :], ones_row[:, :], V[:, :],
                            start=True, stop=True,
                        )
                        vb_sb = smallp.tile([128, K], f32, tag="vb_sb")
                        nc.scalar.copy(out=vb_sb[:, :], in_=ps_b[:, :])

                        # qs = q0*Vb (DVE); w = rowsums(qs) (ACT copy+accum)
                        qs = bigp.tile([128, NT, K], f32, tag="qs")
                        w_new = smallp.tile([128, NT], f32, tag="w_new")
                        for i in range(NT):
                            nc.vector.tensor_mul(
                                out=qs[:, i, :], in0=q0[:, i, :], in1=vb_sb[:, :]
                            )
                            scr3 = smallp.tile([128, K], f32, tag="wacc_scr")
                            nc.scalar.activation(
                                out=scr3[:, :], in_=qs[:, i, :], func=ACT_COPY,
                                accum_out=w_new[:, i:i + 1],
                            )
                        w_cur = w_new

                    # final row normalization: Uf = U / (U*w + eps)
                    t1 = smallp.tile([128, NT], f32, tag="t1")
                    nc.vector.tensor_mul(
                        out=t1[:, :], in0=U[:, :], in1=w_cur[:, :]
                    )
                    t2 = smallp.tile([128, NT], f32, tag="t2")
                    nc.vector.tensor_scalar_add(
                        out=t2[:, :], in0=t1[:, :], scalar1=EPS
                    )
                    t3 = smallp.tile([128, NT], f32, tag="t3")
                    nc.vector.reciprocal(out=t3[:, :], in_=t2[:, :])
                    Uf = smallp.tile([128, NT], f32, tag="Uf")
                    nc.vector.tensor_mul(out=Uf[:, :], in0=U[:, :], in1=t3[:, :])

                    # a = qs * Uf  (qs = q0*V5 from the last iteration)
                    a_sb = bigp.tile([128, NT, K], f32, tag="a_sb")
                    for i in range(NT):
                        nc.vector.tensor_scalar_mul(
                            out=a_sb[:, i, :], in0=qs[:, i, :],
                            scalar1=Uf[:, i:i + 1],
                        )
                    nc.sync.dma_start(
                        out=a_o[mod][b].rearrange("(i p) k -> p i k", p=128),
                        in_=a_sb[:, :, :],
                    )

                    # V in column layout: diag via ident mask + ACT row-sum
                    Vc = smallp.tile([128, KC], f32, tag="Vc")
                    for kk in range(KC):
                        scr4 = smallp.tile([128, 128], f32, tag="vdiag_scr")
                        nc.vector.tensor_mul(
                            out=scr4[:, :],
                            in0=vb_sb[:, kk * 128:(kk + 1) * 128],
                            in1=ident[:, :],
                        )
                        scr5 = smallp.tile([128, 128], f32, tag="vdiag_scr2")
                        nc.scalar.activation(
                            out=scr5[:, :], in_=scr4[:, :], func=ACT_COPY,
                            accum_out=Vc[:, kk:kk + 1],
                        )
                    # pp[k,d] = V[k] * p_norm[k,d]
                    pp = smallp.tile([128, KC, D], f32, tag="pp")
                    for kk in range(KC):
                        nc.vector.tensor_scalar_mul(
                            out=pp[:, kk, :], in0=p_norm[mod][:, kk, :],
                            scalar1=Vc[:, kk:kk + 1],
                        )
                    # z = Uf * (q0 @ pp): q0T blocks stationary
                    z_sb = bigp.tile([128, NT, D], f32, tag="z_sb")
                    for i in range(NT):
                        ps = ps_mm.tile([128, D], f32, tag="mm")
                        for kk in range(KC):
                            nc.tensor.matmul(
                                ps[:, :],
                                q0T[:, kk, i * 128:(i + 1) * 128],
                                pp[:, kk, :],
                                start=(kk == 0), stop=(kk == KC - 1),
                            )
                        nc.scalar.activation(
                            out=z_sb[:, i, :], in_=ps[:, :], func=ACT_COPY,
                            scale=Uf[:, i:i + 1],
                        )
                    nc.sync.dma_start(
                        out=z_o[mod][b].rearrange("(i p) d -> p i d", p=128),
                        in_=z_sb[:, :, :],
                    )

    nc.compile()
    return nc


_NC_CACHE = {}


def _get_nc():
    if "nc" not in _NC_CACHE:
        _NC_CACHE["nc"] = _build()
    return _NC_CACHE["nc"]


def kernel(f_rgb, f_sn, proto_rgb, proto_sn):
    f_rgb = np.ascontiguousarray(f_rgb, dtype=np.float32)
    f_sn = np.ascontiguousarray(f_sn, dtype=np.float32)
    proto_rgb = np.ascontiguousarray(proto_rgb, dtype=np.float32)
    proto_sn = np.ascontiguousarray(proto_sn, dtype=np.float32)

    nc = _get_nc()
    in_maps = []
    for c in range(NCORES):
        sl = slice(c * BPC, (c + 1) * BPC)
        in_maps.append({
            "x_rgb": f_rgb[sl], "x_sn": f_sn[sl],
            "p_rgb": proto_rgb, "p_sn": proto_sn,
        })
    results = run_bass_kernel_spmd(nc, in_maps, list(range(NCORES))).results

    def gather(name):
        return np.concatenate([results[c][name] for c in range(NCORES)], axis=0)

    z_rgb = gather("z_rgb")
    z_sn = gather("z_sn")
    a_rgb = gather("a_rgb")
    a_sn = gather("a_sn")
    sim_rgb = gather("sim_rgb")
    sim_sn = gather("sim_sn")

    cr = a_rgb.sum(axis=1, dtype=np.float64)    # [B, K]
    cs = a_sn.sum(axis=1, dtype=np.float64)
    m = float((cr * cs).sum() / (B * N * N))
    mc = min(max(m, 0.0), 1.0)
    sem = np.float32(0.5 * ((1.0 - mc) + (1.0 - mc)))

    return (z_rgb, z_sn, a_rgb, a_sn, sim_rgb, sim_sn, sem)
